# revision 3
# baseline (speedup 1.0000x reference)
"""CrossAttention kernel for Trainium2 (8 NeuronCores, SPMD).

Problem: x[4,4096,512], context[4,1024,768], Wq[512,512], Wk[768,512],
Wv[768,512], Wo[512,512], bo[512]; 8 heads x 64 dhead; out [4,4096,512].

Sharding: each core takes one (batch, query-half) pair -> fully
independent shards, no collectives. Core c: b = c//2, query rows
[(c%2)*2048 : (c%2)*2048+2048].

Per-core dataflow (all matmuls fp32r = full-rate fp32 on the PE):
  ctx -> (PE transpose) ctxT[cdim,nk]
  kT[inner,nk]   = Wk.T @ ctxT       (lhsT=Wk tiles, rhs=ctxT)
  v'[nk,520]     = ctxT.T @ Wv       (+ ones column per head for the
                                      softmax denominator)
  per 512-row block:
    xT[qdim,rows]  (PE transpose)
    qT[inner,rows] = Wq.T @ xT
    per head h:
      simT[nk,rows] = kT_h.T @ qT_h       (K=64, 8 nk-tiles, PSUM)
      expT = exp(0.125*simT)              (ACT, PSUM->SBUF, fp32r out)
      av[0:65,rows] = v'_h.T @ expT       (accumulate 8 k-tiles;
                                           row 64 = softmax denominator)
    per head pair: reciprocal(den) -> K=2 PE broadcast -> DVE copy,
      outT_h = av_h[0:64] * den_b         (DVE, psum x sbuf)
    yT = Wo.T @ outT + bo ; y = transpose(yT) -> DMA out
"""
import numpy as np
import concourse.bass as bass
import concourse.mybir as mybir
import concourse.tile as tile
from concourse import bacc
from concourse.bass_utils import run_bass_kernel_spmd
from concourse.masks import make_identity
from contextlib import ExitStack

P = 128
B, NQ, NK = 4, 4096, 1024
QD, CD = 512, 768
H, D = 8, 64
IN = H * D          # 512
ROWS = NQ // 2      # 2048 rows per core
RB = 512            # row block
NRB = ROWS // RB    # 4
KT = NK // P        # 8 nk partition-tiles
CT = CD // P        # 6
QT = QD // P        # 4
IO = IN // P        # 4
VW = D + 1          # 65: v columns + ones column per head

FP32 = mybir.dt.float32
FP32R = mybir.dt.float32r
EXP = mybir.ActivationFunctionType.Exp

_CACHE = {}


def _build():
    nc = bacc.Bacc("TRN2", target_bir_lowering=False, debug=False, num_devices=8)
    xs_d = nc.dram_tensor("xs", [ROWS, QD], FP32, kind="ExternalInput").ap()
    ctx_d = nc.dram_tensor("ctx", [NK, CD], FP32, kind="ExternalInput").ap()
    wq_d = nc.dram_tensor("Wq", [QD, IN], FP32, kind="ExternalInput").ap()
    wk_d = nc.dram_tensor("Wk", [CD, IN], FP32, kind="ExternalInput").ap()
    wv_d = nc.dram_tensor("Wv", [CD, IN], FP32, kind="ExternalInput").ap()
    wo_d = nc.dram_tensor("Wo", [IN, QD], FP32, kind="ExternalInput").ap()
    bo_d = nc.dram_tensor("bo", [QD], FP32, kind="ExternalInput").ap()
    y_d = nc.dram_tensor("y", [ROWS, QD], FP32, kind="ExternalOutput").ap()

    with tile.TileContext(nc) as tc, ExitStack() as ctx:
        # ---- pools -------------------------------------------------------
        consts = ctx.enter_context(tc.tile_pool(name="consts", bufs=1))
        persist = ctx.enter_context(tc.tile_pool(name="persist", bufs=1))
        # PSUM: 8 banks total: sim 4 + av 2 + misc 2
        ps_sim = ctx.enter_context(tc.tile_pool(name="ps_sim", bufs=4, space="PSUM"))
        ps_av = ctx.enter_context(tc.tile_pool(name="ps_av", bufs=2, space="PSUM"))
        ps_misc = ctx.enter_context(tc.tile_pool(name="ps_misc", bufs=2, space="PSUM"))

        # ---- constants ---------------------------------------------------
        wq = consts.tile([P, QT, IN], FP32R, tag="wq")
        nc.sync.dma_start(wq[:], wq_d.rearrange("(t p) i -> p t i", p=P).bitcast(FP32R))
        wk = consts.tile([P, CT, IN], FP32R, tag="wk")
        nc.sync.dma_start(wk[:], wk_d.rearrange("(t p) i -> p t i", p=P).bitcast(FP32R))
        wv = consts.tile([P, CT, IN], FP32R, tag="wv")
        nc.sync.dma_start(wv[:], wv_d.rearrange("(t p) i -> p t i", p=P).bitcast(FP32R))
        wo = consts.tile([P, IO, QD], FP32R, tag="wo")
        nc.sync.dma_start(wo[:], wo_d.rearrange("(t p) q -> p t q", p=P).bitcast(FP32R))
        bo32 = consts.tile([P, QT], FP32, tag="bo32")
        nc.sync.dma_start(bo32[:], bo_d.rearrange("(t p) -> p t", p=P))

        ident32 = consts.tile([P, P], FP32, tag="ident32")
        make_identity(nc, ident32[:])
        identr = consts.tile([P, P], FP32R, tag="identr")
        nc.vector.tensor_copy(identr[:], ident32[:])

        ones64f = consts.tile([1, D], FP32, tag="ones64f")
        nc.vector.memset(ones64f[:], 1.0)
        ones64 = consts.tile([1, D], FP32R, tag="ones64")
        nc.vector.tensor_copy(ones64[:], ones64f[:])

        ones8 = consts.tile([P, H], FP32, tag="ones8")
        nc.vector.memset(ones8[:], 1.0)

        # ---- persistent per-core tensors --------------------------------
        kT = persist.tile([P, IO, NK], FP32R, tag="kT")
        vP = persist.tile([P, KT, H * VW], FP32R, tag="vP")

        # ---- phase B: context transpose ---------------------------------
        with tc.tile_pool(name="early", bufs=1) as early:
            ctx_nat = early.tile([P, KT, CD], FP32R, tag="ctx_nat")
            nc.sync.dma_start(
                ctx_nat[:], ctx_d.rearrange("(t p) c -> p t c", p=P).bitcast(FP32R)
            )
            ctxT = early.tile([P, CT, NK], FP32R, tag="ctxT")
            for ct in range(CT):
                for t in range(KT):
                    pt = ps_misc.tile([P, P], FP32R, tag="misc", name=f"trc_{ct}_{t}")
                    nc.tensor.transpose(
                        pt[:], ctx_nat[:, t, ct * P : (ct + 1) * P], identr[:]
                    )
                    nc.vector.tensor_copy(
                        ctxT[:, ct, t * P : (t + 1) * P], pt[:].bitcast(FP32)
                    )

            # ---- phase C: k/v projections -------------------------------
            for ti in range(IO):
                for nh in range(2):
                    ps = ps_misc.tile([P, RB], FP32, tag="misc", name=f"kp_{ti}_{nh}")
                    for ct in range(CT):
                        nc.tensor.matmul(
                            ps[:],
                            wk[:, ct, ti * P : (ti + 1) * P],
                            ctxT[:, ct, nh * RB : (nh + 1) * RB],
                            start=(ct == 0),
                            stop=(ct == CT - 1),
                        )
                    nc.vector.tensor_copy(kT[:, ti, nh * RB : (nh + 1) * RB], ps[:])

            for kt in range(KT):
                ps = ps_misc.tile([P, IN], FP32, tag="misc", name=f"vp_{kt}")
                for ct in range(CT):
                    nc.tensor.matmul(
                        ps[:],
                        ctxT[:, ct, kt * P : (kt + 1) * P],
                        wv[:, ct, :],
                        start=(ct == 0),
                        stop=(ct == CT - 1),
                    )
                vslice = vP[:, kt, :].rearrange("p (h e) -> p h e", e=VW)
                nc.vector.tensor_copy(
                    vslice[:, :, 0:D], ps[:].rearrange("p (h e) -> p h e", e=D)
                )
                nc.vector.tensor_copy(vslice[:, :, D : D + 1], ones8[:, :, None])

        # ---- steady-state pools -----------------------------------------
        p_xn = ctx.enter_context(tc.tile_pool(name="p_xn", bufs=1))
        p_xT = ctx.enter_context(tc.tile_pool(name="p_xT", bufs=2))
        p_qT = ctx.enter_context(tc.tile_pool(name="p_qT", bufs=2))
        p_exp = ctx.enter_context(tc.tile_pool(name="p_exp", bufs=2))
        p_out = ctx.enter_context(tc.tile_pool(name="p_out", bufs=2))
        p_den = ctx.enter_context(tc.tile_pool(name="p_den", bufs=2))
        p_yT = ctx.enter_context(tc.tile_pool(name="p_yT", bufs=1))
        p_yN = ctx.enter_context(tc.tile_pool(name="p_yN", bufs=2))

        for rb in range(NRB):
            r0 = rb * RB
            # x chunk -> SBUF (natural), transpose to xT
            x_nat = p_xn.tile([P, RB // P, QD], FP32R, tag="x_nat")
            nc.sync.dma_start(
                x_nat[:],
                xs_d[r0 : r0 + RB, :].rearrange("(t p) q -> p t q", p=P).bitcast(FP32R),
            )
            xT = p_xT.tile([P, QT, RB], FP32R, tag="xT")
            for t in range(RB // P):
                for qc in range(QT):
                    pt = ps_misc.tile([P, P], FP32R, tag="misc", name=f"trx_{rb}_{t}_{qc}")
                    nc.tensor.transpose(
                        pt[:], x_nat[:, t, qc * P : (qc + 1) * P], identr[:]
                    )
                    nc.vector.tensor_copy(
                        xT[:, qc, t * P : (t + 1) * P], pt[:].bitcast(FP32)
                    )

            # qT = Wq.T @ xT
            qT = p_qT.tile([P, IO, RB], FP32R, tag="qT")
            for ti in range(IO):
                ps = ps_misc.tile([P, RB], FP32, tag="misc", name=f"qp_{rb}_{ti}")
                for qc in range(QT):
                    nc.tensor.matmul(
                        ps[:],
                        wq[:, qc, ti * P : (ti + 1) * P],
                        xT[:, qc, :],
                        start=(qc == 0),
                        stop=(qc == QT - 1),
                    )
                nc.vector.tensor_copy(qT[:, ti, :], ps[:])

            outT = p_out.tile([P, IO, RB], FP32R, tag="outT")

            # head loop, software-pipelined by one head on the PE
            pend = None  # (h, expt)

            def emit_tail(h):
                # AV + softmax-normalize for head h (its expT is complete)
                io, po = h // 2, (h % 2) * D
                av = ps_av.tile([P, RB], FP32, tag="av", name=f"av_{rb}_{h}")
                for kt in range(KT):
                    nc.tensor.matmul(
                        av[0:VW, :],
                        vP[:, kt, h * VW : (h + 1) * VW],
                        pend[1][:, kt, :],
                        start=(kt == 0),
                        stop=(kt == KT - 1),
                    )
                den_inv = p_den.tile([1, RB], FP32R, tag="den_inv")
                with nc.allow_low_precision(reason="fp32r reciprocal"):
                    nc.vector.reciprocal(den_inv[0:1, :], av[D : D + 1, :])
                db_ps = ps_misc.tile([P, RB], FP32, tag="misc", name=f"db_{rb}_{h}")
                nc.tensor.matmul(db_ps[0:D, :], ones64[:], den_inv[0:1, :])
                db = p_den.tile([D, RB], FP32, tag="den_b")
                nc.vector.tensor_copy(db[:], db_ps[0:D, :])
                nc.vector.tensor_tensor(
                    outT[po : po + D, io, :],
                    av[0:D, :],
                    db[0:D, :],
                    mybir.AluOpType.mult,
                )

            for h in range(H):
                io, po = h // 2, (h % 2) * D
                expt = p_exp.tile([P, KT, RB], FP32R, tag="expT")
                for kt in range(KT):
                    sp = ps_sim.tile([P, RB], FP32, tag="sim", name=f"sim_{rb}_{h}_{kt}")
                    nc.tensor.matmul(
                        sp[:],
                        kT[po : po + D, io, kt * P : (kt + 1) * P],
                        qT[po : po + D, io, :],
                    )
                    nc.scalar.activation(expt[:, kt, :], sp[:], EXP, scale=0.125)
                if pend is not None:
                    emit_tail(pend[0])
                pend = (h, expt)

            emit_tail(pend[0])

            # y^T = Wo.T @ outT + bo
            yT = p_yT.tile([P, QT, RB], FP32R, tag="yT")
            for qo in range(QT):
                ps = ps_misc.tile([P, RB], FP32, tag="misc", name=f"yp_{rb}_{qo}")
                for ti in range(IO):
                    nc.tensor.matmul(
                        ps[:],
                        wo[:, ti, qo * P : (qo + 1) * P],
                        outT[:, ti, :],
                        start=(ti == 0),
                        stop=(ti == IO - 1),
                    )
                nc.vector.tensor_scalar_add(yT[:, qo, :], ps[:], bo32[:, qo : qo + 1])

            # transpose back to natural rows and DMA out
            yN = p_yN.tile([P, RB // P, QD], FP32, tag="yN")
            for qo in range(QT):
                for t in range(RB // P):
                    pt = ps_misc.tile([P, P], FP32R, tag="misc", name=f"try_{rb}_{qo}_{t}")
                    nc.tensor.transpose(
                        pt[:], yT[:, qo, t * P : (t + 1) * P], identr[:]
                    )
                    nc.vector.tensor_copy(
                        yN[:, t, qo * P : (qo + 1) * P], pt[:].bitcast(FP32)
                    )
            nc.sync.dma_start(
                y_d[r0 : r0 + RB, :].rearrange("(t p) q -> p t q", p=P), yN[:]
            )

    nc.compile()
    return nc


def kernel(x, context, Wq, Wk, Wv, Wo, bo):
    x = np.ascontiguousarray(np.asarray(x, dtype=np.float32))
    context = np.ascontiguousarray(np.asarray(context, dtype=np.float32))
    Wq = np.ascontiguousarray(np.asarray(Wq, dtype=np.float32))
    Wk = np.ascontiguousarray(np.asarray(Wk, dtype=np.float32))
    Wv = np.ascontiguousarray(np.asarray(Wv, dtype=np.float32))
    Wo = np.ascontiguousarray(np.asarray(Wo, dtype=np.float32))
    bo = np.ascontiguousarray(np.asarray(bo, dtype=np.float32))

    if "nc" not in _CACHE:
        _CACHE["nc"] = _build()
    nc = _CACHE["nc"]

    in_maps = []
    for c in range(8):
        b, half = c // 2, c % 2
        in_maps.append({
            "xs": np.ascontiguousarray(x[b, half * ROWS : (half + 1) * ROWS]),
            "ctx": context[b],
            "Wq": Wq, "Wk": Wk, "Wv": Wv, "Wo": Wo, "bo": bo,
        })

    res = run_bass_kernel_spmd(nc, in_maps, core_ids=list(range(8)))
    out = np.empty((B, NQ, QD), dtype=np.float32)
    for c in range(8):
        b, half = c // 2, c % 2
        out[b, half * ROWS : (half + 1) * ROWS] = res.results[c]["y"]
    return out
